# revision 5
# baseline (speedup 1.0000x reference)
"""Trainium2 Bass kernel for the EnergyBasedModel relaxation problem.

Math (per batch row, 20 sequential steps, STEP_SIZE=0.005):
  s1 <- 1.005*s1 - 0.005*dsig(s1) * (sig(x)@w0 + sig(s2)@w1.T + b0)
  s2 <- 1.005*s2 - 0.005*dsig(s2) * (sig(s1)@w1 + sig(s3)@w2.T + b1)
  s3 <- 1.005*s3 - 0.005*dsig(s3) * (sig(s2)@w2 + b2)
  return s3

Strategy:
  - Data-parallel over the 4096-row batch across 8 cores (512 rows each).
  - States held transposed in SBUF: [features, batch] so that both big
    matmuls per step use weight chunks as the stationary (lhsT) operand
    with batch (512) as the moving free dim.
  - sig(x) @ w0 + b0 is constant across steps: precomputed once on device.
  - w1 is needed in both orientations (contract over either axis). Both
    orientations are streamed from DRAM each step, double-buffered; host
    pre-chunks them into contiguous per-column-block layout.
"""

import os
import numpy as np
import ml_dtypes

import concourse.bacc as bacc
import concourse.tile as tile
from concourse import mybir
from concourse.bass_utils import run_bass_kernel_spmd

N_CORES = 8
BATCH = 4096
B = BATCH // N_CORES          # 512 rows per core
D0, D1, D2, D3 = 1024, 2048, 2048, 10
NC0 = D0 // 128               # 8 chunks
NC1 = D1 // 128               # 16 chunks
N_STEPS = 20
LAM = 0.1 / 20                # 0.005
F32 = mybir.dt.float32

# matmul dtype: "f32" (exact, 4x slower PE), "f32r" (fast fp32, reduced
# precision multiply), "bf16" (fast, bf16 storage for weights/activations)
MM_MODE = os.environ.get("EBM_MM_MODE", "bf16")


def _mm_dt():
    return {"f32": mybir.dt.float32, "f32r": mybir.dt.float32, "bf16": mybir.dt.bfloat16}[MM_MODE]


def _np_mm_dt():
    return {"f32": np.float32, "f32r": np.float32, "bf16": ml_dtypes.bfloat16}[MM_MODE]


def _mm_cast(ap):
    """View an AP in the dtype the matmul should run in."""
    if MM_MODE == "f32r":
        return ap.bitcast(mybir.dt.float32r)
    return ap


def _build():
    nc = bacc.Bacc("TRN2", target_bir_lowering=False, debug=False, num_devices=N_CORES)
    mm = _mm_dt()
    ACT = mybir.ActivationFunctionType
    ALU = mybir.AluOpType

    xT_d = nc.dram_tensor("xT", [D0, B], F32, kind="ExternalInput")
    w0p_d = nc.dram_tensor("w0p", [NC1, 128, D0], mm, kind="ExternalInput")
    w1p_d = nc.dram_tensor("w1p", [NC1, 128, D1], mm, kind="ExternalInput")
    w1tp_d = nc.dram_tensor("w1tp", [NC1, 128, D1], mm, kind="ExternalInput")
    w2p_d = nc.dram_tensor("w2p", [128, NC1 * D3], mm, kind="ExternalInput")
    w2tp_d = nc.dram_tensor("w2tp", [D3, D1], mm, kind="ExternalInput")
    b0p_d = nc.dram_tensor("b0p", [128, NC1], F32, kind="ExternalInput")
    b1p_d = nc.dram_tensor("b1p", [128, NC1], F32, kind="ExternalInput")
    b2p_d = nc.dram_tensor("b2p", [D3, 1], F32, kind="ExternalInput")
    s1t_d = nc.dram_tensor("s1t", [D1, B], F32, kind="ExternalInput")
    s2t_d = nc.dram_tensor("s2t", [D1, B], F32, kind="ExternalInput")
    s3t_d = nc.dram_tensor("s3t", [D3, B], F32, kind="ExternalInput")
    out_d = nc.dram_tensor("out", [D3, B], F32, kind="ExternalOutput")

    wcol_bufs = 4 if MM_MODE == "bf16" else 2
    ew_bufs = 3

    with tile.TileContext(nc) as tc:
        with (
            tc.tile_pool(name="persist", bufs=1) as per,
            tc.tile_pool(name="psum", bufs=6, space="PSUM") as psum,
            tc.tile_pool(name="psum3", bufs=2, space="PSUM") as psum3,
            tc.tile_pool(name="wstream", bufs=wcol_bufs) as wstream,
            tc.tile_pool(name="ew", bufs=ew_bufs) as ew,
        ):
            s1sb = per.tile([128, NC1 * B], F32)
            s2sb = per.tile([128, NC1 * B], F32)
            s3sb = per.tile([D3, B], F32)
            c1sb = per.tile([128, NC1 * B], F32)
            g1sb = per.tile([128, NC1 * B], mm)
            g2sb = per.tile([128, NC1 * B], mm)
            g3sb = per.tile([D3, B], mm)
            w2sb = per.tile([128, NC1 * D3], mm)
            w2tsb = per.tile([D3, D1], mm)
            b1sb = per.tile([128, NC1], F32)
            b2sb = per.tile([D3, 1], F32)

            def col(m):
                return slice(m * B, (m + 1) * B)

            # ---- initial loads ----
            for m in range(NC1):
                nc.sync.dma_start(s1sb[:, col(m)], s1t_d[m * 128:(m + 1) * 128, :])
                nc.sync.dma_start(s2sb[:, col(m)], s2t_d[m * 128:(m + 1) * 128, :])
            nc.sync.dma_start(s3sb[:], s3t_d[:])
            nc.sync.dma_start(w2sb[:], w2p_d[:])
            nc.sync.dma_start(w2tsb[:], w2tp_d[:])
            nc.sync.dma_start(b1sb[:], b1p_d[:])
            nc.sync.dma_start(b2sb[:], b2p_d[:])

            # initial sigmoids of the states
            for m in range(NC1):
                nc.scalar.activation(g1sb[:, col(m)], s1sb[:, col(m)], ACT.Sigmoid)
                nc.scalar.activation(g2sb[:, col(m)], s2sb[:, col(m)], ACT.Sigmoid)
            nc.scalar.activation(g3sb[:], s3sb[:], ACT.Sigmoid)

            # ---- precompute C1 = sig(x) @ w0 + b0 (transposed) ----
            with tc.tile_pool(name="pre", bufs=1) as prepool:
                sx = prepool.tile([128, NC0 * B], mm)
                b0sb = prepool.tile([128, NC1], F32)
                nc.sync.dma_start(b0sb[:], b0p_d[:])
                for k in range(NC0):
                    xt_t = ew.tile([128, B], F32, tag="pre")
                    nc.sync.dma_start(xt_t[:], xT_d[k * 128:(k + 1) * 128, :])
                    nc.scalar.activation(sx[:, col(k)], xt_t[:], ACT.Sigmoid)
                for m in range(NC1):
                    wcol = wstream.tile([128, D1], mm, tag="wcol")
                    nc.sync.dma_start(wcol[:, :D0], w0p_d[m])
                    pt = psum.tile([128, B], F32, tag="pt")
                    for k in range(NC0):
                        nc.tensor.matmul(
                            pt[:],
                            _mm_cast(wcol[:, k * 128:(k + 1) * 128]),
                            _mm_cast(sx[:, col(k)]),
                            start=(k == 0),
                            stop=(k == NC0 - 1),
                        )
                    nc.vector.tensor_scalar_add(c1sb[:, col(m)], pt[:], b0sb[:, m:m + 1])

            # ---- relaxation loop ----
            def update(mcol, pre_t, s_ap, g_ap):
                """State update for one [P, B] chunk given pre-activation tile.

                s <- 1.005*s - LAM*sig(s)*(1-sig(s))*pre ; g <- sig(s_new)
                """
                gt = ew.tile(list(pre_t.shape), F32, tag="gt")
                nc.scalar.activation(gt[:], s_ap, ACT.Sigmoid)
                # gt <- (gt - 1) * gt  == -sig*(1-sig)
                nc.vector.scalar_tensor_tensor(gt[:], gt[:], 1.0, gt[:], op0=ALU.subtract, op1=ALU.mult)
                # pre_t <- (gt * LAM) * pre_t  == -LAM*dsig*pre
                nc.vector.scalar_tensor_tensor(pre_t[:], gt[:], LAM, pre_t[:], op0=ALU.mult, op1=ALU.mult)
                # s <- 1.005*s + pre_t
                nc.vector.scalar_tensor_tensor(s_ap, s_ap, 1.0 + LAM, pre_t[:], op0=ALU.mult, op1=ALU.add)
                nc.scalar.activation(g_ap, s_ap, ACT.Sigmoid)

            for _step in range(N_STEPS):
                # phase A: layer-1 update. pre1 = C1 + w1T-matmul(g2)
                for m in range(NC1):
                    wcol = wstream.tile([128, D1], mm, tag="wcol")
                    nc.sync.dma_start(wcol[:], w1tp_d[m])
                    pt = psum.tile([128, B], F32, tag="pt")
                    for k in range(NC1):
                        nc.tensor.matmul(
                            pt[:],
                            _mm_cast(wcol[:, k * 128:(k + 1) * 128]),
                            _mm_cast(g2sb[:, col(k)]),
                            start=(k == 0),
                            stop=(k == NC1 - 1),
                        )
                    pre_t = ew.tile([128, B], F32, tag="pre")
                    nc.vector.tensor_add(pre_t[:], pt[:], c1sb[:, col(m)])
                    update(col(m), pre_t, s1sb[:, col(m)], g1sb[:, col(m)])

                # phase B: layer-2 update. pre2 = w1-matmul(g1) + w2T-matmul(g3) + b1
                for m in range(NC1):
                    wcol = wstream.tile([128, D1], mm, tag="wcol")
                    nc.sync.dma_start(wcol[:], w1p_d[m])
                    pt = psum.tile([128, B], F32, tag="pt")
                    for k in range(NC1):
                        nc.tensor.matmul(
                            pt[:],
                            _mm_cast(wcol[:, k * 128:(k + 1) * 128]),
                            _mm_cast(g1sb[:, col(k)]),
                            start=(k == 0),
                            stop=False,
                        )
                    nc.tensor.matmul(
                        pt[:],
                        _mm_cast(w2tsb[:, m * 128:(m + 1) * 128]),
                        _mm_cast(g3sb[:]),
                        start=False,
                        stop=True,
                    )
                    pre_t = ew.tile([128, B], F32, tag="pre")
                    nc.vector.tensor_scalar_add(pre_t[:], pt[:], b1sb[:, m:m + 1])
                    update(col(m), pre_t, s2sb[:, col(m)], g2sb[:, col(m)])

                # phase C: layer-3 (output) update. pre3 = w2-matmul(g2) + b2
                pt3 = psum3.tile([D3, B], F32, tag="pt3")
                for k in range(NC1):
                    nc.tensor.matmul(
                        pt3[:],
                        _mm_cast(w2sb[:, k * D3:(k + 1) * D3]),
                        _mm_cast(g2sb[:, col(k)]),
                        start=(k == 0),
                        stop=(k == NC1 - 1),
                    )
                pre3 = ew.tile([D3, B], F32, tag="pre")
                nc.vector.tensor_scalar_add(pre3[:], pt3[:], b2sb[:])
                update(None, pre3, s3sb[:], g3sb[:])

            nc.sync.dma_start(out_d[:], s3sb[:])

    nc.compile()
    return nc


_NC_CACHE = {}


def _get_nc():
    if MM_MODE not in _NC_CACHE:
        _NC_CACHE[MM_MODE] = _build()
    return _NC_CACHE[MM_MODE]


def _prep_weights(w0, w1, w2, b0, b1, b2):
    np_mm = _np_mm_dt()
    w0p = np.ascontiguousarray(
        w0.reshape(NC0, 128, NC1, 128).transpose(2, 1, 0, 3).reshape(NC1, 128, D0)
    ).astype(np_mm)
    w1p = np.ascontiguousarray(
        w1.reshape(NC1, 128, NC1, 128).transpose(2, 1, 0, 3).reshape(NC1, 128, D1)
    ).astype(np_mm)
    w1tp = np.ascontiguousarray(
        w1.reshape(NC1, 128, NC1, 128).transpose(0, 3, 2, 1).reshape(NC1, 128, D1)
    ).astype(np_mm)
    w2p = np.ascontiguousarray(
        w2.reshape(NC1, 128, D3).transpose(1, 0, 2).reshape(128, NC1 * D3)
    ).astype(np_mm)
    w2tp = np.ascontiguousarray(w2.T).astype(np_mm)
    b0p = np.ascontiguousarray(b0.reshape(NC1, 128).T).astype(np.float32)
    b1p = np.ascontiguousarray(b1.reshape(NC1, 128).T).astype(np.float32)
    b2p = b2.reshape(D3, 1).astype(np.float32)
    return dict(w0p=w0p, w1p=w1p, w1tp=w1tp, w2p=w2p, w2tp=w2tp,
                b0p=b0p, b1p=b1p, b2p=b2p)


def _run(inputs, trace=False, trace_kwargs=None):
    x = np.asarray(inputs["x"], np.float32)
    s1 = np.asarray(inputs["s1"], np.float32)
    s2 = np.asarray(inputs["s2"], np.float32)
    s3 = np.asarray(inputs["s3"], np.float32)
    shared = _prep_weights(
        np.asarray(inputs["w0"], np.float32), np.asarray(inputs["w1"], np.float32),
        np.asarray(inputs["w2"], np.float32), np.asarray(inputs["b0"], np.float32),
        np.asarray(inputs["b1"], np.float32), np.asarray(inputs["b2"], np.float32))

    in_maps = []
    for c in range(N_CORES):
        rows = slice(c * B, (c + 1) * B)
        m = dict(shared)
        m["xT"] = np.ascontiguousarray(x[rows].T)
        m["s1t"] = np.ascontiguousarray(s1[rows].T)
        m["s2t"] = np.ascontiguousarray(s2[rows].T)
        m["s3t"] = np.ascontiguousarray(s3[rows].T)
        in_maps.append(m)

    nc = _get_nc()
    kw = {}
    if trace:
        kw = dict(trace=True, trace_kwargs=trace_kwargs or {})
    res = run_bass_kernel_spmd(nc, in_maps, list(range(N_CORES)), **kw)
    out = np.empty((BATCH, D3), np.float32)
    for c in range(N_CORES):
        out[c * B:(c + 1) * B, :] = res.results[c]["out"].T
    return out, res


def kernel(**inputs) -> np.ndarray:
    out, _ = _run(inputs)
    return out


def _make_in_maps(inputs):
    x = np.asarray(inputs["x"], np.float32)
    s1 = np.asarray(inputs["s1"], np.float32)
    s2 = np.asarray(inputs["s2"], np.float32)
    s3 = np.asarray(inputs["s3"], np.float32)
    shared = _prep_weights(
        np.asarray(inputs["w0"], np.float32), np.asarray(inputs["w1"], np.float32),
        np.asarray(inputs["w2"], np.float32), np.asarray(inputs["b0"], np.float32),
        np.asarray(inputs["b1"], np.float32), np.asarray(inputs["b2"], np.float32))
    in_maps = []
    for c in range(N_CORES):
        rows = slice(c * B, (c + 1) * B)
        m = dict(shared)
        m["xT"] = np.ascontiguousarray(x[rows].T)
        m["s1t"] = np.ascontiguousarray(s1[rows].T)
        m["s2t"] = np.ascontiguousarray(s2[rows].T)
        m["s3t"] = np.ascontiguousarray(s3[rows].T)
        in_maps.append(m)
    return in_maps


def timed_run(inputs, iters=5):
    """Run the kernel with device-resident inputs, timing each execution.

    Returns (output [4096,10], list of per-iteration wall seconds).
    """
    import time
    import jax
    from jax.sharding import Mesh, PartitionSpec, NamedSharding
    from jax.experimental.shard_map import shard_map
    from concourse import mybir as _mybir
    from concourse.bass2jax import _bass_exec_p, install_neuronx_cc_hook, partition_id_tensor

    install_neuronx_cc_hook()
    nc = _get_nc()
    in_maps = _make_in_maps(inputs)

    partition_name = nc.partition_id_tensor.name if nc.partition_id_tensor else None
    in_names, out_names, out_avals, zero_outs = [], [], [], []
    for alloc in nc.m.functions[0].allocations:
        if not isinstance(alloc, _mybir.MemoryLocationSet):
            continue
        name = alloc.memorylocations[0].name
        if alloc.kind == "ExternalInput":
            if name != partition_name:
                in_names.append(name)
        elif alloc.kind == "ExternalOutput":
            shape = tuple(alloc.tensor_shape)
            dtype = _mybir.dt.np(alloc.dtype)
            out_names.append(name)
            out_avals.append(jax.core.ShapedArray(shape, dtype))
            zero_outs.append(np.zeros(shape, dtype))
    n_params = len(in_names)
    all_in = list(in_names) + list(out_names)
    if partition_name is not None:
        all_in.append(partition_name)
    donate = tuple(range(n_params, n_params + len(out_names)))

    def _body(*args):
        operands = list(args)
        if partition_name is not None:
            operands.append(partition_id_tensor())
        outs = _bass_exec_p.bind(
            *operands,
            out_avals=tuple(out_avals),
            in_names=tuple(all_in),
            out_names=tuple(out_names),
            lowering_input_output_aliases=(),
            sim_require_finite=True,
            sim_require_nnan=True,
            nc=nc,
        )
        return tuple(outs)

    devices = jax.devices()[:N_CORES]
    mesh = Mesh(np.asarray(devices), ("core",))
    spec = PartitionSpec("core")
    sharded = jax.jit(
        shard_map(_body, mesh=mesh, in_specs=(spec,) * (n_params + len(out_names)),
                  out_specs=(spec,) * len(out_names), check_rep=False),
        donate_argnums=donate, keep_unused=True)

    concat_in = [
        np.concatenate([np.asarray(in_maps[c][nm]) for c in range(N_CORES)], axis=0)
        for nm in in_names
    ]
    sh = NamedSharding(mesh, spec)
    dev_in = [jax.device_put(a, sh) for a in concat_in]
    concat_zeros = [np.zeros((N_CORES * z.shape[0], *z.shape[1:]), z.dtype) for z in zero_outs]

    times = []
    out_arrs = None
    for it in range(iters + 1):
        zs = [jax.device_put(z, sh) for z in concat_zeros]
        jax.block_until_ready(zs)
        t0 = time.perf_counter()
        out_arrs = sharded(*dev_in, *zs)
        jax.block_until_ready(out_arrs)
        dt = time.perf_counter() - t0
        if it > 0:
            times.append(dt)

    res0 = np.asarray(out_arrs[0]).reshape(N_CORES, *out_avals[0].shape)
    out = np.empty((BATCH, D3), np.float32)
    for c in range(N_CORES):
        out[c * B:(c + 1) * B, :] = res0[c].T
    return out, times


# revision 8
# speedup vs baseline: 1.1272x; 1.1272x over previous
"""Trainium2 Bass kernel for the EnergyBasedModel relaxation problem.

Math (per batch row, 20 sequential steps, STEP_SIZE=0.005):
  s1 <- 1.005*s1 - 0.005*dsig(s1) * (sig(x)@w0 + sig(s2)@w1.T + b0)
  s2 <- 1.005*s2 - 0.005*dsig(s2) * (sig(s1)@w1 + sig(s3)@w2.T + b1)
  s3 <- 1.005*s3 - 0.005*dsig(s3) * (sig(s2)@w2 + b2)
  return s3

Strategy:
  - Data-parallel over the 4096-row batch across 8 cores (512 rows each).
  - States held transposed in SBUF: [features, batch] so that both big
    matmuls per step use weight chunks as the stationary (lhsT) operand
    with batch (512) as the moving free dim.
  - sig(x) @ w0 + b0 is constant across steps: precomputed once on device.
  - w1 is needed in both orientations (contract over either axis). Both
    orientations are streamed from DRAM each step, double-buffered; host
    pre-chunks them into contiguous per-column-block layout.
"""

import os
import numpy as np
import ml_dtypes

import concourse.bacc as bacc
import concourse.tile as tile
from concourse import mybir
from concourse.bass_utils import run_bass_kernel_spmd

N_CORES = 8
BATCH = 4096
B = BATCH // N_CORES          # 512 rows per core
D0, D1, D2, D3 = 1024, 2048, 2048, 10
NC0 = D0 // 128               # 8 chunks
NC1 = D1 // 128               # 16 chunks
N_STEPS = 20
LAM = 0.1 / 20                # 0.005
F32 = mybir.dt.float32

# matmul dtype: "f32" (exact, 4x slower PE), "f32r" (fast fp32, reduced
# precision multiply), "bf16" (fast, bf16 storage for weights/activations)
MM_MODE = os.environ.get("EBM_MM_MODE", "bf16")


def _mm_dt():
    return {"f32": mybir.dt.float32, "f32r": mybir.dt.float32r, "bf16": mybir.dt.bfloat16}[MM_MODE]


def _np_mm_dt():
    return {"f32": np.float32, "f32r": np.float32, "bf16": ml_dtypes.bfloat16}[MM_MODE]


def _mm_cast(ap):
    return ap


def _build():
    nc = bacc.Bacc("TRN2", target_bir_lowering=False, debug=False, num_devices=N_CORES)
    mm = _mm_dt()
    ACT = mybir.ActivationFunctionType
    ALU = mybir.AluOpType

    xT_d = nc.dram_tensor("xT", [D0, B], F32, kind="ExternalInput")
    w0p_d = nc.dram_tensor("w0p", [NC1, 128, D0], mm, kind="ExternalInput")
    w1p_d = nc.dram_tensor("w1p", [NC1, 128, D1], mm, kind="ExternalInput")
    w1tp_d = nc.dram_tensor("w1tp", [NC1, 128, D1], mm, kind="ExternalInput")
    w2p_d = nc.dram_tensor("w2p", [128, NC1 * D3], mm, kind="ExternalInput")
    w2tp_d = nc.dram_tensor("w2tp", [D3, D1], mm, kind="ExternalInput")
    b0p_d = nc.dram_tensor("b0p", [128, NC1], F32, kind="ExternalInput")
    b1p_d = nc.dram_tensor("b1p", [128, NC1], F32, kind="ExternalInput")
    b2p_d = nc.dram_tensor("b2p", [D3, 1], F32, kind="ExternalInput")
    s1t_d = nc.dram_tensor("s1t", [D1, B], F32, kind="ExternalInput")
    s2t_d = nc.dram_tensor("s2t", [D1, B], F32, kind="ExternalInput")
    s3t_d = nc.dram_tensor("s3t", [D3, B], F32, kind="ExternalInput")
    out_d = nc.dram_tensor("out", [D3, B], F32, kind="ExternalOutput")

    wcol_bufs = 4 if MM_MODE == "bf16" else 2
    ew_bufs = 3 if MM_MODE == "bf16" else 2

    with tile.TileContext(nc) as tc:
        with (
            tc.tile_pool(name="persist", bufs=1) as per,
            tc.tile_pool(name="psum", bufs=6, space="PSUM") as psum,
            tc.tile_pool(name="psum3", bufs=2, space="PSUM") as psum3,
            tc.tile_pool(name="wstream", bufs=wcol_bufs) as wstream,
            tc.tile_pool(name="ew", bufs=ew_bufs) as ew,
        ):
            s1sb = per.tile([128, NC1 * B], F32)
            s2sb = per.tile([128, NC1 * B], F32)
            s3sb = per.tile([D3, B], F32)
            c1sb = per.tile([128, NC1 * B], F32)
            g1sb = per.tile([128, NC1 * B], mm)
            g2sb = per.tile([128, NC1 * B], mm)
            g3sb = per.tile([D3, B], mm)
            w2sb = per.tile([128, NC1 * D3], mm)
            w2tsb = per.tile([D3, D1], mm)
            b1sb = per.tile([128, NC1], F32)
            b2sb = per.tile([D3, 1], F32)

            def col(m):
                return slice(m * B, (m + 1) * B)

            # ---- initial loads ----
            for m in range(NC1):
                nc.sync.dma_start(s1sb[:, col(m)], s1t_d[m * 128:(m + 1) * 128, :])
                nc.sync.dma_start(s2sb[:, col(m)], s2t_d[m * 128:(m + 1) * 128, :])
            nc.sync.dma_start(s3sb[:], s3t_d[:])
            nc.sync.dma_start(w2sb[:], w2p_d[:])
            nc.sync.dma_start(w2tsb[:], w2tp_d[:])
            nc.sync.dma_start(b1sb[:], b1p_d[:])
            nc.sync.dma_start(b2sb[:], b2p_d[:])

            # initial sigmoids of the states
            for m in range(NC1):
                nc.scalar.activation(g1sb[:, col(m)], s1sb[:, col(m)], ACT.Sigmoid)
                nc.scalar.activation(g2sb[:, col(m)], s2sb[:, col(m)], ACT.Sigmoid)
            nc.scalar.activation(g3sb[:], s3sb[:], ACT.Sigmoid)

            # ---- precompute C1 = sig(x) @ w0 + b0 (transposed) ----
            # Done in two half-passes over the D0 contraction dim to halve
            # the sig(x) staging buffer (SBUF is tight in f32 mode).
            NH = NC0 // 2
            with tc.tile_pool(name="pre", bufs=1) as prepool:
                sx = prepool.tile([128, NH * B], mm)
                b0sb = prepool.tile([128, NC1], F32)
                nc.sync.dma_start(b0sb[:], b0p_d[:])
                for half in range(2):
                    for kk in range(NH):
                        k = half * NH + kk
                        xt_t = ew.tile([128, B], F32, tag="pre")
                        nc.sync.dma_start(xt_t[:], xT_d[k * 128:(k + 1) * 128, :])
                        nc.scalar.activation(sx[:, col(kk)], xt_t[:], ACT.Sigmoid)
                    for m in range(NC1):
                        wcol = wstream.tile([128, D1], mm, tag="wcol")
                        nc.sync.dma_start(
                            wcol[:, :NH * 128],
                            w0p_d[m][:, half * NH * 128:(half + 1) * NH * 128],
                        )
                        pt = psum.tile([128, B], F32, tag="pt")
                        for kk in range(NH):
                            nc.tensor.matmul(
                                pt[:],
                                _mm_cast(wcol[:, kk * 128:(kk + 1) * 128]),
                                _mm_cast(sx[:, col(kk)]),
                                start=(kk == 0),
                                stop=(kk == NH - 1),
                            )
                        if half == 0:
                            nc.vector.tensor_scalar_add(c1sb[:, col(m)], pt[:], b0sb[:, m:m + 1])
                        else:
                            nc.vector.tensor_add(c1sb[:, col(m)], pt[:], c1sb[:, col(m)])

            # ---- relaxation loop ----
            def update(mcol, pre_t, s_ap, g_ap):
                """State update for one [P, B] chunk given pre-activation tile.

                s <- 1.005*s - LAM*sig(s)*(1-sig(s))*pre ; g <- sig(s_new)
                """
                gt = ew.tile(list(pre_t.shape), F32, tag="gt")
                nc.scalar.activation(gt[:], s_ap, ACT.Sigmoid)
                # gt <- (gt - 1) * gt  == -sig*(1-sig)
                nc.vector.scalar_tensor_tensor(gt[:], gt[:], 1.0, gt[:], op0=ALU.subtract, op1=ALU.mult)
                # pre_t <- (gt * LAM) * pre_t  == -LAM*dsig*pre
                nc.vector.scalar_tensor_tensor(pre_t[:], gt[:], LAM, pre_t[:], op0=ALU.mult, op1=ALU.mult)
                # s <- 1.005*s + pre_t
                nc.vector.scalar_tensor_tensor(s_ap, s_ap, 1.0 + LAM, pre_t[:], op0=ALU.mult, op1=ALU.add)
                nc.scalar.activation(g_ap, s_ap, ACT.Sigmoid)

            for _step in range(N_STEPS):
                # phase A: layer-1 update. pre1 = C1 + w1T-matmul(g2)
                for m in range(NC1):
                    wcol = wstream.tile([128, D1], mm, tag="wcol")
                    nc.sync.dma_start(wcol[:], w1tp_d[m])
                    pt = psum.tile([128, B], F32, tag="pt")
                    for k in range(NC1):
                        nc.tensor.matmul(
                            pt[:],
                            _mm_cast(wcol[:, k * 128:(k + 1) * 128]),
                            _mm_cast(g2sb[:, col(k)]),
                            start=(k == 0),
                            stop=(k == NC1 - 1),
                        )
                    pre_t = ew.tile([128, B], F32, tag="pre")
                    nc.vector.tensor_add(pre_t[:], pt[:], c1sb[:, col(m)])
                    update(col(m), pre_t, s1sb[:, col(m)], g1sb[:, col(m)])

                # phase B: layer-2 update. pre2 = w1-matmul(g1) + w2T-matmul(g3) + b1
                for m in range(NC1):
                    wcol = wstream.tile([128, D1], mm, tag="wcol")
                    nc.sync.dma_start(wcol[:], w1p_d[m])
                    pt = psum.tile([128, B], F32, tag="pt")
                    for k in range(NC1):
                        nc.tensor.matmul(
                            pt[:],
                            _mm_cast(wcol[:, k * 128:(k + 1) * 128]),
                            _mm_cast(g1sb[:, col(k)]),
                            start=(k == 0),
                            stop=False,
                        )
                    nc.tensor.matmul(
                        pt[:],
                        _mm_cast(w2tsb[:, m * 128:(m + 1) * 128]),
                        _mm_cast(g3sb[:]),
                        start=False,
                        stop=True,
                    )
                    pre_t = ew.tile([128, B], F32, tag="pre")
                    nc.vector.tensor_scalar_add(pre_t[:], pt[:], b1sb[:, m:m + 1])
                    update(col(m), pre_t, s2sb[:, col(m)], g2sb[:, col(m)])

                # phase C: layer-3 (output) update. pre3 = w2-matmul(g2) + b2
                pt3 = psum3.tile([D3, B], F32, tag="pt3")
                for k in range(NC1):
                    nc.tensor.matmul(
                        pt3[:],
                        _mm_cast(w2sb[:, k * D3:(k + 1) * D3]),
                        _mm_cast(g2sb[:, col(k)]),
                        start=(k == 0),
                        stop=(k == NC1 - 1),
                    )
                pre3 = ew.tile([D3, B], F32, tag="pre")
                nc.vector.tensor_scalar_add(pre3[:], pt3[:], b2sb[:])
                update(None, pre3, s3sb[:], g3sb[:])

            nc.sync.dma_start(out_d[:], s3sb[:])

    nc.compile()
    return nc


_NC_CACHE = {}


def _get_nc():
    if MM_MODE not in _NC_CACHE:
        _NC_CACHE[MM_MODE] = _build()
    return _NC_CACHE[MM_MODE]


def _prep_weights(w0, w1, w2, b0, b1, b2):
    np_mm = _np_mm_dt()
    w0p = np.ascontiguousarray(
        w0.reshape(NC0, 128, NC1, 128).transpose(2, 1, 0, 3).reshape(NC1, 128, D0)
    ).astype(np_mm)
    w1p = np.ascontiguousarray(
        w1.reshape(NC1, 128, NC1, 128).transpose(2, 1, 0, 3).reshape(NC1, 128, D1)
    ).astype(np_mm)
    w1tp = np.ascontiguousarray(
        w1.reshape(NC1, 128, NC1, 128).transpose(0, 3, 2, 1).reshape(NC1, 128, D1)
    ).astype(np_mm)
    w2p = np.ascontiguousarray(
        w2.reshape(NC1, 128, D3).transpose(1, 0, 2).reshape(128, NC1 * D3)
    ).astype(np_mm)
    w2tp = np.ascontiguousarray(w2.T).astype(np_mm)
    b0p = np.ascontiguousarray(b0.reshape(NC1, 128).T).astype(np.float32)
    b1p = np.ascontiguousarray(b1.reshape(NC1, 128).T).astype(np.float32)
    b2p = b2.reshape(D3, 1).astype(np.float32)
    return dict(w0p=w0p, w1p=w1p, w1tp=w1tp, w2p=w2p, w2tp=w2tp,
                b0p=b0p, b1p=b1p, b2p=b2p)


def _run(inputs, trace=False, trace_kwargs=None):
    x = np.asarray(inputs["x"], np.float32)
    s1 = np.asarray(inputs["s1"], np.float32)
    s2 = np.asarray(inputs["s2"], np.float32)
    s3 = np.asarray(inputs["s3"], np.float32)
    shared = _prep_weights(
        np.asarray(inputs["w0"], np.float32), np.asarray(inputs["w1"], np.float32),
        np.asarray(inputs["w2"], np.float32), np.asarray(inputs["b0"], np.float32),
        np.asarray(inputs["b1"], np.float32), np.asarray(inputs["b2"], np.float32))

    in_maps = []
    for c in range(N_CORES):
        rows = slice(c * B, (c + 1) * B)
        m = dict(shared)
        m["xT"] = np.ascontiguousarray(x[rows].T)
        m["s1t"] = np.ascontiguousarray(s1[rows].T)
        m["s2t"] = np.ascontiguousarray(s2[rows].T)
        m["s3t"] = np.ascontiguousarray(s3[rows].T)
        in_maps.append(m)

    nc = _get_nc()
    kw = {}
    if trace:
        kw = dict(trace=True, trace_kwargs=trace_kwargs or {})
    res = run_bass_kernel_spmd(nc, in_maps, list(range(N_CORES)), **kw)
    out = np.empty((BATCH, D3), np.float32)
    for c in range(N_CORES):
        out[c * B:(c + 1) * B, :] = res.results[c]["out"].T
    return out, res


def kernel(**inputs) -> np.ndarray:
    out, _ = _run(inputs)
    return out


def _make_in_maps(inputs):
    x = np.asarray(inputs["x"], np.float32)
    s1 = np.asarray(inputs["s1"], np.float32)
    s2 = np.asarray(inputs["s2"], np.float32)
    s3 = np.asarray(inputs["s3"], np.float32)
    shared = _prep_weights(
        np.asarray(inputs["w0"], np.float32), np.asarray(inputs["w1"], np.float32),
        np.asarray(inputs["w2"], np.float32), np.asarray(inputs["b0"], np.float32),
        np.asarray(inputs["b1"], np.float32), np.asarray(inputs["b2"], np.float32))
    in_maps = []
    for c in range(N_CORES):
        rows = slice(c * B, (c + 1) * B)
        m = dict(shared)
        m["xT"] = np.ascontiguousarray(x[rows].T)
        m["s1t"] = np.ascontiguousarray(s1[rows].T)
        m["s2t"] = np.ascontiguousarray(s2[rows].T)
        m["s3t"] = np.ascontiguousarray(s3[rows].T)
        in_maps.append(m)
    return in_maps


def timed_run(inputs, iters=5):
    """Run the kernel with device-resident inputs, timing each execution.

    Returns (output [4096,10], list of per-iteration wall seconds).
    """
    import time
    import jax
    from jax.sharding import Mesh, PartitionSpec, NamedSharding
    from jax.experimental.shard_map import shard_map
    from concourse import mybir as _mybir
    from concourse.bass2jax import _bass_exec_p, install_neuronx_cc_hook, partition_id_tensor

    install_neuronx_cc_hook()
    nc = _get_nc()
    in_maps = _make_in_maps(inputs)

    partition_name = nc.partition_id_tensor.name if nc.partition_id_tensor else None
    in_names, out_names, out_avals, zero_outs = [], [], [], []
    for alloc in nc.m.functions[0].allocations:
        if not isinstance(alloc, _mybir.MemoryLocationSet):
            continue
        name = alloc.memorylocations[0].name
        if alloc.kind == "ExternalInput":
            if name != partition_name:
                in_names.append(name)
        elif alloc.kind == "ExternalOutput":
            shape = tuple(alloc.tensor_shape)
            dtype = _mybir.dt.np(alloc.dtype)
            out_names.append(name)
            out_avals.append(jax.core.ShapedArray(shape, dtype))
            zero_outs.append(np.zeros(shape, dtype))
    n_params = len(in_names)
    all_in = list(in_names) + list(out_names)
    if partition_name is not None:
        all_in.append(partition_name)
    donate = tuple(range(n_params, n_params + len(out_names)))

    def _body(*args):
        operands = list(args)
        if partition_name is not None:
            operands.append(partition_id_tensor())
        outs = _bass_exec_p.bind(
            *operands,
            out_avals=tuple(out_avals),
            in_names=tuple(all_in),
            out_names=tuple(out_names),
            lowering_input_output_aliases=(),
            sim_require_finite=True,
            sim_require_nnan=True,
            nc=nc,
        )
        return tuple(outs)

    devices = jax.devices()[:N_CORES]
    mesh = Mesh(np.asarray(devices), ("core",))
    spec = PartitionSpec("core")
    sharded = jax.jit(
        shard_map(_body, mesh=mesh, in_specs=(spec,) * (n_params + len(out_names)),
                  out_specs=(spec,) * len(out_names), check_rep=False),
        donate_argnums=donate, keep_unused=True)

    concat_in = [
        np.concatenate([np.asarray(in_maps[c][nm]) for c in range(N_CORES)], axis=0)
        for nm in in_names
    ]
    sh = NamedSharding(mesh, spec)
    dev_in = [jax.device_put(a, sh) for a in concat_in]
    concat_zeros = [np.zeros((N_CORES * z.shape[0], *z.shape[1:]), z.dtype) for z in zero_outs]

    times = []
    out_arrs = None
    for it in range(iters + 1):
        zs = [jax.device_put(z, sh) for z in concat_zeros]
        jax.block_until_ready(zs)
        t0 = time.perf_counter()
        out_arrs = sharded(*dev_in, *zs)
        jax.block_until_ready(out_arrs)
        dt = time.perf_counter() - t0
        if it > 0:
            times.append(dt)

    res0 = np.asarray(out_arrs[0]).reshape(N_CORES, *out_avals[0].shape)
    out = np.empty((BATCH, D3), np.float32)
    for c in range(N_CORES):
        out[c * B:(c + 1) * B, :] = res0[c].T
    return out, times


# revision 12
# speedup vs baseline: 1.2533x; 1.1119x over previous
"""Trainium2 Bass kernel for the EnergyBasedModel relaxation problem.

Math (per batch row, 20 sequential steps, STEP_SIZE=0.005):
  s1 <- 1.005*s1 - 0.005*dsig(s1) * (sig(x)@w0 + sig(s2)@w1.T + b0)
  s2 <- 1.005*s2 - 0.005*dsig(s2) * (sig(s1)@w1 + sig(s3)@w2.T + b1)
  s3 <- 1.005*s3 - 0.005*dsig(s3) * (sig(s2)@w2 + b2)
  return s3

Strategy:
  - Data-parallel over the 4096-row batch across 8 cores (512 rows each).
  - States held transposed in SBUF: [features, batch] so that both big
    matmuls per step use weight chunks as the stationary (lhsT) operand
    with batch (512) as the moving free dim.
  - sig(x) @ w0 + b0 is constant across steps: precomputed once on device.
  - w1 is needed in both orientations (contract over either axis). Both
    orientations are streamed from DRAM each step, double-buffered; host
    pre-chunks them into contiguous per-column-block layout.
"""

import os
import numpy as np
import ml_dtypes

import concourse.bacc as bacc
import concourse.tile as tile
from concourse import mybir
from concourse.bass_utils import run_bass_kernel_spmd

N_CORES = 8
BATCH = 4096
B = BATCH // N_CORES          # 512 rows per core
D0, D1, D2, D3 = 1024, 2048, 2048, 10
NC0 = D0 // 128               # 8 chunks
NC1 = D1 // 128               # 16 chunks
N_STEPS = 20
LAM = 0.1 / 20                # 0.005
F32 = mybir.dt.float32

# matmul dtype: "f32" (exact, 4x slower PE), "f32r" (fast fp32, reduced
# precision multiply), "bf16" (fast, bf16 storage for weights/activations)
MM_MODE = os.environ.get("EBM_MM_MODE", "bf16")


def _mm_dt():
    return {"f32": mybir.dt.float32, "f32r": mybir.dt.float32r, "bf16": mybir.dt.bfloat16}[MM_MODE]


def _np_mm_dt():
    return {"f32": np.float32, "f32r": np.float32, "bf16": ml_dtypes.bfloat16}[MM_MODE]


def _mm_cast(ap):
    return ap


def _build():
    nc = bacc.Bacc("TRN2", target_bir_lowering=False, debug=False, num_devices=N_CORES)
    mm = _mm_dt()
    ACT = mybir.ActivationFunctionType
    ALU = mybir.AluOpType

    xT_d = nc.dram_tensor("xT", [D0, B], F32, kind="ExternalInput")
    w0p_d = nc.dram_tensor("w0p", [NC1, 128, D0], mm, kind="ExternalInput")
    w1p_d = nc.dram_tensor("w1p", [NC1, 128, D1], mm, kind="ExternalInput")
    w1tp_d = nc.dram_tensor("w1tp", [NC1, 128, D1], mm, kind="ExternalInput")
    w2p_d = nc.dram_tensor("w2p", [128, NC1 * D3], mm, kind="ExternalInput")
    w2tp_d = nc.dram_tensor("w2tp", [D3, D1], mm, kind="ExternalInput")
    b0p_d = nc.dram_tensor("b0p", [128, NC1], F32, kind="ExternalInput")
    b1p_d = nc.dram_tensor("b1p", [128, NC1], F32, kind="ExternalInput")
    b2p_d = nc.dram_tensor("b2p", [D3, 1], F32, kind="ExternalInput")
    s1t_d = nc.dram_tensor("s1t", [D1, B], F32, kind="ExternalInput")
    s2t_d = nc.dram_tensor("s2t", [D1, B], F32, kind="ExternalInput")
    s3t_d = nc.dram_tensor("s3t", [D3, B], F32, kind="ExternalInput")
    out_d = nc.dram_tensor("out", [D3, B], F32, kind="ExternalOutput")

    wcol_bufs = 4 if MM_MODE == "bf16" else 3
    ew_bufs = 3 if MM_MODE == "bf16" else 2

    with tile.TileContext(nc) as tc:
        with (
            tc.tile_pool(name="persist", bufs=1) as per,
            tc.tile_pool(name="psum", bufs=6, space="PSUM") as psum,
            tc.tile_pool(name="psum3", bufs=2, space="PSUM") as psum3,
            tc.tile_pool(name="wstream", bufs=wcol_bufs) as wstream,
            tc.tile_pool(name="ew", bufs=ew_bufs) as ew,
        ):
            s1sb = per.tile([128, NC1 * B], F32)
            s2sb = per.tile([128, NC1 * B], F32)
            s3sb = per.tile([D3, B], F32)
            c1sb = per.tile([128, NC1 * B], F32)
            g1sb = per.tile([128, NC1 * B], mm)
            g2sb = per.tile([128, NC1 * B], mm)
            g3sb = per.tile([D3, B], mm)
            w2sb = per.tile([128, NC1 * D3], mm)
            b1sb = per.tile([128, NC1], F32)
            b2sb = per.tile([D3, 1], F32)

            def col(m):
                return slice(m * B, (m + 1) * B)

            # ---- initial loads ----
            # s2 first: the step-0 phase-A matmuls only need g2 = sig(s2), so
            # prioritizing it lets compute start while s1 still streams in.
            for m in range(NC1):
                nc.sync.dma_start(s2sb[:, col(m)], s2t_d[m * 128:(m + 1) * 128, :])
                nc.scalar.activation(g2sb[:, col(m)], s2sb[:, col(m)], ACT.Sigmoid)
            nc.sync.dma_start(s3sb[:], s3t_d[:])
            nc.scalar.activation(g3sb[:], s3sb[:], ACT.Sigmoid)
            nc.sync.dma_start(w2sb[:], w2p_d[:])
            nc.sync.dma_start(b1sb[:], b1p_d[:])
            nc.sync.dma_start(b2sb[:], b2p_d[:])
            for m in range(NC1):
                nc.sync.dma_start(s1sb[:, col(m)], s1t_d[m * 128:(m + 1) * 128, :])
                nc.scalar.activation(g1sb[:, col(m)], s1sb[:, col(m)], ACT.Sigmoid)

            # ---- precompute C1 = sig(x) @ w0 + b0 (transposed) ----
            # Done in two half-passes over the D0 contraction dim to halve
            # the sig(x) staging buffer (SBUF is tight in f32 mode).
            NH = NC0 // 2
            with tc.tile_pool(name="pre", bufs=1) as prepool:
                sx = prepool.tile([128, NH * B], mm)
                b0sb = prepool.tile([128, NC1], F32)
                nc.sync.dma_start(b0sb[:], b0p_d[:])
                for half in range(2):
                    for kk in range(NH):
                        k = half * NH + kk
                        xt_t = ew.tile([128, B], F32, tag="pre")
                        nc.sync.dma_start(xt_t[:], xT_d[k * 128:(k + 1) * 128, :])
                        nc.scalar.activation(sx[:, col(kk)], xt_t[:], ACT.Sigmoid)
                    for m in range(NC1):
                        wcol = wstream.tile([128, D1], mm, tag="wcol")
                        nc.sync.dma_start(
                            wcol[:, :NH * 128],
                            w0p_d[m][:, half * NH * 128:(half + 1) * NH * 128],
                        )
                        pt = psum.tile([128, B], F32, tag="pt")
                        for kk in range(NH):
                            nc.tensor.matmul(
                                pt[:],
                                _mm_cast(wcol[:, kk * 128:(kk + 1) * 128]),
                                _mm_cast(sx[:, col(kk)]),
                                start=(kk == 0),
                                stop=(kk == NH - 1),
                            )
                        if half == 0:
                            nc.vector.tensor_scalar_add(c1sb[:, col(m)], pt[:], b0sb[:, m:m + 1])
                        else:
                            nc.vector.tensor_add(c1sb[:, col(m)], pt[:], c1sb[:, col(m)])

            # ---- relaxation loop ----
            def update(mcol, pre_t, s_ap, g_ap):
                """State update for one [P, B] chunk given pre-activation tile.

                s <- 1.005*s - LAM*sig(s)*(1-sig(s))*pre ; g <- sig(s_new)
                """
                gt = ew.tile(list(pre_t.shape), F32, tag="gt")
                nc.scalar.activation(gt[:], s_ap, ACT.Sigmoid)
                # gt <- (gt - 1) * gt  == -sig*(1-sig)
                nc.vector.scalar_tensor_tensor(gt[:], gt[:], 1.0, gt[:], op0=ALU.subtract, op1=ALU.mult)
                # pre_t <- (gt * LAM) * pre_t  == -LAM*dsig*pre
                nc.vector.scalar_tensor_tensor(pre_t[:], gt[:], LAM, pre_t[:], op0=ALU.mult, op1=ALU.mult)
                # s <- 1.005*s + pre_t
                nc.vector.scalar_tensor_tensor(s_ap, s_ap, 1.0 + LAM, pre_t[:], op0=ALU.mult, op1=ALU.add)
                nc.scalar.activation(g_ap, s_ap, ACT.Sigmoid)

            for _step in range(N_STEPS):
                # phase A: layer-1 update. pre1 = C1 + w1T-matmul(g2)
                for m in range(NC1):
                    wcol = wstream.tile([128, D1], mm, tag="wcol")
                    nc.sync.dma_start(wcol[:], w1tp_d[m])
                    pt = psum.tile([128, B], F32, tag="pt")
                    for k in range(NC1):
                        nc.tensor.matmul(
                            pt[:],
                            _mm_cast(wcol[:, k * 128:(k + 1) * 128]),
                            _mm_cast(g2sb[:, col(k)]),
                            start=(k == 0),
                            stop=(k == NC1 - 1),
                        )
                    pre_t = ew.tile([128, B], F32, tag="pre")
                    nc.vector.tensor_add(pre_t[:], pt[:], c1sb[:, col(m)])
                    update(col(m), pre_t, s1sb[:, col(m)], g1sb[:, col(m)])

                # phase B: layer-2 update. pre2 = w1-matmul(g1) + w2T-matmul(g3) + b1
                for m in range(NC1):
                    wcol = wstream.tile([128, D1], mm, tag="wcol")
                    nc.sync.dma_start(wcol[:], w1p_d[m])
                    pt = psum.tile([128, B], F32, tag="pt")
                    for k in range(NC1):
                        nc.tensor.matmul(
                            pt[:],
                            _mm_cast(wcol[:, k * 128:(k + 1) * 128]),
                            _mm_cast(g1sb[:, col(k)]),
                            start=(k == 0),
                            stop=False,
                        )
                    w2t_t = wstream.tile([D3, 128], mm, tag="w2t")
                    nc.sync.dma_start(w2t_t[:], w2tp_d[:, m * 128:(m + 1) * 128])
                    nc.tensor.matmul(
                        pt[:],
                        _mm_cast(w2t_t[:]),
                        _mm_cast(g3sb[:]),
                        start=False,
                        stop=True,
                    )
                    pre_t = ew.tile([128, B], F32, tag="pre")
                    nc.vector.tensor_scalar_add(pre_t[:], pt[:], b1sb[:, m:m + 1])
                    update(col(m), pre_t, s2sb[:, col(m)], g2sb[:, col(m)])

                # phase C: layer-3 (output) update. pre3 = w2-matmul(g2) + b2
                pt3 = psum3.tile([D3, B], F32, tag="pt3")
                for k in range(NC1):
                    nc.tensor.matmul(
                        pt3[:],
                        _mm_cast(w2sb[:, k * D3:(k + 1) * D3]),
                        _mm_cast(g2sb[:, col(k)]),
                        start=(k == 0),
                        stop=(k == NC1 - 1),
                    )
                pre3 = ew.tile([D3, B], F32, tag="pre")
                nc.vector.tensor_scalar_add(pre3[:], pt3[:], b2sb[:])
                update(None, pre3, s3sb[:], g3sb[:])

            nc.sync.dma_start(out_d[:], s3sb[:])

    nc.compile()
    return nc


_NC_CACHE = {}


def _get_nc():
    if MM_MODE not in _NC_CACHE:
        _NC_CACHE[MM_MODE] = _build()
    return _NC_CACHE[MM_MODE]


def _prep_weights(w0, w1, w2, b0, b1, b2):
    np_mm = _np_mm_dt()
    w0p = np.ascontiguousarray(
        w0.reshape(NC0, 128, NC1, 128).transpose(2, 1, 0, 3).reshape(NC1, 128, D0)
    ).astype(np_mm)
    w1p = np.ascontiguousarray(
        w1.reshape(NC1, 128, NC1, 128).transpose(2, 1, 0, 3).reshape(NC1, 128, D1)
    ).astype(np_mm)
    w1tp = np.ascontiguousarray(
        w1.reshape(NC1, 128, NC1, 128).transpose(0, 3, 2, 1).reshape(NC1, 128, D1)
    ).astype(np_mm)
    w2p = np.ascontiguousarray(
        w2.reshape(NC1, 128, D3).transpose(1, 0, 2).reshape(128, NC1 * D3)
    ).astype(np_mm)
    w2tp = np.ascontiguousarray(w2.T).astype(np_mm)
    b0p = np.ascontiguousarray(b0.reshape(NC1, 128).T).astype(np.float32)
    b1p = np.ascontiguousarray(b1.reshape(NC1, 128).T).astype(np.float32)
    b2p = b2.reshape(D3, 1).astype(np.float32)
    return dict(w0p=w0p, w1p=w1p, w1tp=w1tp, w2p=w2p, w2tp=w2tp,
                b0p=b0p, b1p=b1p, b2p=b2p)


def _run(inputs, trace=False, trace_kwargs=None):
    x = np.asarray(inputs["x"], np.float32)
    s1 = np.asarray(inputs["s1"], np.float32)
    s2 = np.asarray(inputs["s2"], np.float32)
    s3 = np.asarray(inputs["s3"], np.float32)
    shared = _prep_weights(
        np.asarray(inputs["w0"], np.float32), np.asarray(inputs["w1"], np.float32),
        np.asarray(inputs["w2"], np.float32), np.asarray(inputs["b0"], np.float32),
        np.asarray(inputs["b1"], np.float32), np.asarray(inputs["b2"], np.float32))

    in_maps = []
    for c in range(N_CORES):
        rows = slice(c * B, (c + 1) * B)
        m = dict(shared)
        m["xT"] = np.ascontiguousarray(x[rows].T)
        m["s1t"] = np.ascontiguousarray(s1[rows].T)
        m["s2t"] = np.ascontiguousarray(s2[rows].T)
        m["s3t"] = np.ascontiguousarray(s3[rows].T)
        in_maps.append(m)

    nc = _get_nc()
    kw = {}
    if trace:
        kw = dict(trace=True, trace_kwargs=trace_kwargs or {})
    res = run_bass_kernel_spmd(nc, in_maps, list(range(N_CORES)), **kw)
    out = np.empty((BATCH, D3), np.float32)
    for c in range(N_CORES):
        out[c * B:(c + 1) * B, :] = res.results[c]["out"].T
    return out, res


def kernel(**inputs) -> np.ndarray:
    out, _ = _run(inputs)
    return out


def _make_in_maps(inputs):
    x = np.asarray(inputs["x"], np.float32)
    s1 = np.asarray(inputs["s1"], np.float32)
    s2 = np.asarray(inputs["s2"], np.float32)
    s3 = np.asarray(inputs["s3"], np.float32)
    shared = _prep_weights(
        np.asarray(inputs["w0"], np.float32), np.asarray(inputs["w1"], np.float32),
        np.asarray(inputs["w2"], np.float32), np.asarray(inputs["b0"], np.float32),
        np.asarray(inputs["b1"], np.float32), np.asarray(inputs["b2"], np.float32))
    in_maps = []
    for c in range(N_CORES):
        rows = slice(c * B, (c + 1) * B)
        m = dict(shared)
        m["xT"] = np.ascontiguousarray(x[rows].T)
        m["s1t"] = np.ascontiguousarray(s1[rows].T)
        m["s2t"] = np.ascontiguousarray(s2[rows].T)
        m["s3t"] = np.ascontiguousarray(s3[rows].T)
        in_maps.append(m)
    return in_maps


def timed_run(inputs, iters=5):
    """Run the kernel with device-resident inputs, timing each execution.

    Returns (output [4096,10], list of per-iteration wall seconds).
    """
    import time
    import jax
    from jax.sharding import Mesh, PartitionSpec, NamedSharding
    from jax.experimental.shard_map import shard_map
    from concourse import mybir as _mybir
    from concourse.bass2jax import _bass_exec_p, install_neuronx_cc_hook, partition_id_tensor

    install_neuronx_cc_hook()
    nc = _get_nc()
    in_maps = _make_in_maps(inputs)

    partition_name = nc.partition_id_tensor.name if nc.partition_id_tensor else None
    in_names, out_names, out_avals, zero_outs = [], [], [], []
    for alloc in nc.m.functions[0].allocations:
        if not isinstance(alloc, _mybir.MemoryLocationSet):
            continue
        name = alloc.memorylocations[0].name
        if alloc.kind == "ExternalInput":
            if name != partition_name:
                in_names.append(name)
        elif alloc.kind == "ExternalOutput":
            shape = tuple(alloc.tensor_shape)
            dtype = _mybir.dt.np(alloc.dtype)
            out_names.append(name)
            out_avals.append(jax.core.ShapedArray(shape, dtype))
            zero_outs.append(np.zeros(shape, dtype))
    n_params = len(in_names)
    all_in = list(in_names) + list(out_names)
    if partition_name is not None:
        all_in.append(partition_name)
    donate = tuple(range(n_params, n_params + len(out_names)))

    def _body(*args):
        operands = list(args)
        if partition_name is not None:
            operands.append(partition_id_tensor())
        outs = _bass_exec_p.bind(
            *operands,
            out_avals=tuple(out_avals),
            in_names=tuple(all_in),
            out_names=tuple(out_names),
            lowering_input_output_aliases=(),
            sim_require_finite=True,
            sim_require_nnan=True,
            nc=nc,
        )
        return tuple(outs)

    devices = jax.devices()[:N_CORES]
    mesh = Mesh(np.asarray(devices), ("core",))
    spec = PartitionSpec("core")
    sharded = jax.jit(
        shard_map(_body, mesh=mesh, in_specs=(spec,) * (n_params + len(out_names)),
                  out_specs=(spec,) * len(out_names), check_rep=False),
        donate_argnums=donate, keep_unused=True)

    concat_in = [
        np.concatenate([np.asarray(in_maps[c][nm]) for c in range(N_CORES)], axis=0)
        for nm in in_names
    ]
    sh = NamedSharding(mesh, spec)
    dev_in = [jax.device_put(a, sh) for a in concat_in]
    concat_zeros = [np.zeros((N_CORES * z.shape[0], *z.shape[1:]), z.dtype) for z in zero_outs]

    times = []
    out_arrs = None
    for it in range(iters + 1):
        zs = [jax.device_put(z, sh) for z in concat_zeros]
        jax.block_until_ready(zs)
        t0 = time.perf_counter()
        out_arrs = sharded(*dev_in, *zs)
        jax.block_until_ready(out_arrs)
        dt = time.perf_counter() - t0
        if it > 0:
            times.append(dt)

    res0 = np.asarray(out_arrs[0]).reshape(N_CORES, *out_avals[0].shape)
    out = np.empty((BATCH, D3), np.float32)
    for c in range(N_CORES):
        out[c * B:(c + 1) * B, :] = res0[c].T
    return out, times


# revision 14
# speedup vs baseline: 1.2576x; 1.0034x over previous
"""Trainium2 Bass kernel for the EnergyBasedModel relaxation problem.

Math (per batch row, 20 sequential steps, STEP_SIZE=0.005):
  s1 <- 1.005*s1 - 0.005*dsig(s1) * (sig(x)@w0 + sig(s2)@w1.T + b0)
  s2 <- 1.005*s2 - 0.005*dsig(s2) * (sig(s1)@w1 + sig(s3)@w2.T + b1)
  s3 <- 1.005*s3 - 0.005*dsig(s3) * (sig(s2)@w2 + b2)
  return s3

Strategy:
  - Data-parallel over the 4096-row batch across 8 cores (512 rows each).
  - States held transposed in SBUF: [features, batch] so that both big
    matmuls per step use weight chunks as the stationary (lhsT) operand
    with batch (512) as the moving free dim.
  - sig(x) @ w0 + b0 is constant across steps: precomputed once on device.
  - w1 is needed in both orientations (contract over either axis). Both
    orientations are streamed from DRAM each step, double-buffered; host
    pre-chunks them into contiguous per-column-block layout.
"""

import os
import numpy as np
import ml_dtypes

import concourse.bacc as bacc
import concourse.tile as tile
from concourse import mybir
from concourse.bass_utils import run_bass_kernel_spmd

N_CORES = 8
BATCH = 4096
B = BATCH // N_CORES          # 512 rows per core
D0, D1, D2, D3 = 1024, 2048, 2048, 10
NC0 = D0 // 128               # 8 chunks
NC1 = D1 // 128               # 16 chunks
N_STEPS = 20
LAM = 0.1 / 20                # 0.005
F32 = mybir.dt.float32

# matmul dtype: "f32" (exact, 4x slower PE), "f32r" (fast fp32, reduced
# precision multiply — the PE streams it at full rate for N>=256 while
# keeping ~1e-6 relative error on this problem), "bf16" (no faster than
# f32r here, 12x worse error)
MM_MODE = os.environ.get("EBM_MM_MODE", "f32r")


def _mm_dt():
    return {"f32": mybir.dt.float32, "f32r": mybir.dt.float32r, "bf16": mybir.dt.bfloat16}[MM_MODE]


def _np_mm_dt():
    return {"f32": np.float32, "f32r": np.float32, "bf16": ml_dtypes.bfloat16}[MM_MODE]


def _mm_cast(ap):
    return ap


def _build():
    nc = bacc.Bacc("TRN2", target_bir_lowering=False, debug=False, num_devices=N_CORES)
    mm = _mm_dt()
    ACT = mybir.ActivationFunctionType
    ALU = mybir.AluOpType

    xT_d = nc.dram_tensor("xT", [D0, B], F32, kind="ExternalInput")
    w0p_d = nc.dram_tensor("w0p", [NC1, 128, D0], mm, kind="ExternalInput")
    w1p_d = nc.dram_tensor("w1p", [NC1, 128, D1], mm, kind="ExternalInput")
    w1tp_d = nc.dram_tensor("w1tp", [NC1, 128, D1], mm, kind="ExternalInput")
    w2p_d = nc.dram_tensor("w2p", [128, NC1 * D3], mm, kind="ExternalInput")
    w2tp_d = nc.dram_tensor("w2tp", [D3, D1], mm, kind="ExternalInput")
    b0p_d = nc.dram_tensor("b0p", [128, NC1], F32, kind="ExternalInput")
    b1p_d = nc.dram_tensor("b1p", [128, NC1], F32, kind="ExternalInput")
    b2p_d = nc.dram_tensor("b2p", [D3, 1], F32, kind="ExternalInput")
    s1t_d = nc.dram_tensor("s1t", [D1, B], F32, kind="ExternalInput")
    s2t_d = nc.dram_tensor("s2t", [D1, B], F32, kind="ExternalInput")
    s3t_d = nc.dram_tensor("s3t", [D3, B], F32, kind="ExternalInput")
    out_d = nc.dram_tensor("out", [D3, B], F32, kind="ExternalOutput")

    wcol_bufs = 4 if MM_MODE == "bf16" else 3
    ew_bufs = 3 if MM_MODE == "bf16" else 2

    with tile.TileContext(nc) as tc:
        with (
            tc.tile_pool(name="persist", bufs=1) as per,
            tc.tile_pool(name="psum", bufs=6, space="PSUM") as psum,
            tc.tile_pool(name="psum3", bufs=2, space="PSUM") as psum3,
            tc.tile_pool(name="wstream", bufs=wcol_bufs) as wstream,
            tc.tile_pool(name="ew", bufs=ew_bufs) as ew,
        ):
            s1sb = per.tile([128, NC1 * B], F32)
            s2sb = per.tile([128, NC1 * B], F32)
            s3sb = per.tile([D3, B], F32)
            c1sb = per.tile([128, NC1 * B], F32)
            g1sb = per.tile([128, NC1 * B], mm)
            g2sb = per.tile([128, NC1 * B], mm)
            g3sb = per.tile([D3, B], mm)
            w2sb = per.tile([128, NC1 * D3], mm)
            b1sb = per.tile([128, NC1], F32)
            b2sb = per.tile([D3, 1], F32)

            def col(m):
                return slice(m * B, (m + 1) * B)

            # ---- initial loads ----
            # s2 first: the step-0 phase-A matmuls only need g2 = sig(s2), so
            # prioritizing it lets compute start while s1 still streams in.
            for m in range(NC1):
                nc.sync.dma_start(s2sb[:, col(m)], s2t_d[m * 128:(m + 1) * 128, :])
                nc.scalar.activation(g2sb[:, col(m)], s2sb[:, col(m)], ACT.Sigmoid)
            nc.sync.dma_start(s3sb[:], s3t_d[:])
            nc.scalar.activation(g3sb[:], s3sb[:], ACT.Sigmoid)
            nc.sync.dma_start(w2sb[:], w2p_d[:])
            nc.sync.dma_start(b1sb[:], b1p_d[:])
            nc.sync.dma_start(b2sb[:], b2p_d[:])
            for m in range(NC1):
                nc.sync.dma_start(s1sb[:, col(m)], s1t_d[m * 128:(m + 1) * 128, :])
                nc.scalar.activation(g1sb[:, col(m)], s1sb[:, col(m)], ACT.Sigmoid)

            # ---- precompute C1 = sig(x) @ w0 + b0 (transposed) ----
            # Done in two half-passes over the D0 contraction dim to halve
            # the sig(x) staging buffer (SBUF is tight in f32 mode).
            NH = NC0 // 2
            with tc.tile_pool(name="pre", bufs=1) as prepool:
                sx = prepool.tile([128, NH * B], mm)
                b0sb = prepool.tile([128, NC1], F32)
                nc.sync.dma_start(b0sb[:], b0p_d[:])
                for half in range(2):
                    for kk in range(NH):
                        k = half * NH + kk
                        xt_t = ew.tile([128, B], F32, tag="pre")
                        nc.sync.dma_start(xt_t[:], xT_d[k * 128:(k + 1) * 128, :])
                        nc.scalar.activation(sx[:, col(kk)], xt_t[:], ACT.Sigmoid)
                    # 4 weight columns per DMA so the PE groups (16 matmuls,
                    # ~3.4us) outlast the 1MB transfer and its queue latency.
                    for mq in range(NC1 // 4):
                        wcol = wstream.tile([128, D1], mm, tag="wcol")
                        src = w0p_d[mq * 4:(mq + 1) * 4, :,
                                    half * NH * 128:(half + 1) * NH * 128]
                        nc.sync.dma_start(
                            wcol[:].rearrange("p (m e) -> p m e", m=4),
                            src.rearrange("m p e -> p m e"),
                        )
                        for mm_i in range(4):
                            m = mq * 4 + mm_i
                            pt = psum.tile([128, B], F32, tag="pt")
                            for kk in range(NH):
                                nc.tensor.matmul(
                                    pt[:],
                                    _mm_cast(wcol[:, (mm_i * NH + kk) * 128:(mm_i * NH + kk + 1) * 128]),
                                    _mm_cast(sx[:, col(kk)]),
                                    start=(kk == 0),
                                    stop=(kk == NH - 1),
                                )
                            if half == 0:
                                nc.vector.tensor_scalar_add(c1sb[:, col(m)], pt[:], b0sb[:, m:m + 1])
                            else:
                                nc.vector.tensor_add(c1sb[:, col(m)], pt[:], c1sb[:, col(m)])

            # ---- relaxation loop ----
            def update(mcol, pre_t, s_ap, g_ap):
                """State update for one [P, B] chunk given pre-activation tile.

                s <- 1.005*s - LAM*sig(s)*(1-sig(s))*pre ; g <- sig(s_new)
                """
                gt = ew.tile(list(pre_t.shape), F32, tag="gt")
                nc.scalar.activation(gt[:], s_ap, ACT.Sigmoid)
                # gt <- (gt - 1) * gt  == -sig*(1-sig)
                nc.vector.scalar_tensor_tensor(gt[:], gt[:], 1.0, gt[:], op0=ALU.subtract, op1=ALU.mult)
                # pre_t <- (gt * LAM) * pre_t  == -LAM*dsig*pre
                nc.vector.scalar_tensor_tensor(pre_t[:], gt[:], LAM, pre_t[:], op0=ALU.mult, op1=ALU.mult)
                # s <- 1.005*s + pre_t
                nc.vector.scalar_tensor_tensor(s_ap, s_ap, 1.0 + LAM, pre_t[:], op0=ALU.mult, op1=ALU.add)
                nc.scalar.activation(g_ap, s_ap, ACT.Sigmoid)

            for _step in range(N_STEPS):
                # phase A: layer-1 update. pre1 = C1 + w1T-matmul(g2)
                for m in range(NC1):
                    wcol = wstream.tile([128, D1], mm, tag="wcol")
                    nc.sync.dma_start(wcol[:], w1tp_d[m])
                    pt = psum.tile([128, B], F32, tag="pt")
                    for k in range(NC1):
                        nc.tensor.matmul(
                            pt[:],
                            _mm_cast(wcol[:, k * 128:(k + 1) * 128]),
                            _mm_cast(g2sb[:, col(k)]),
                            start=(k == 0),
                            stop=(k == NC1 - 1),
                        )
                    pre_t = ew.tile([128, B], F32, tag="pre")
                    nc.vector.tensor_add(pre_t[:], pt[:], c1sb[:, col(m)])
                    update(col(m), pre_t, s1sb[:, col(m)], g1sb[:, col(m)])

                # phase B: layer-2 update. pre2 = w1-matmul(g1) + w2T-matmul(g3) + b1
                for m in range(NC1):
                    wcol = wstream.tile([128, D1], mm, tag="wcol")
                    nc.sync.dma_start(wcol[:], w1p_d[m])
                    pt = psum.tile([128, B], F32, tag="pt")
                    for k in range(NC1):
                        nc.tensor.matmul(
                            pt[:],
                            _mm_cast(wcol[:, k * 128:(k + 1) * 128]),
                            _mm_cast(g1sb[:, col(k)]),
                            start=(k == 0),
                            stop=False,
                        )
                    w2t_t = wstream.tile([D3, 128], mm, tag="w2t")
                    nc.sync.dma_start(w2t_t[:], w2tp_d[:, m * 128:(m + 1) * 128])
                    nc.tensor.matmul(
                        pt[:],
                        _mm_cast(w2t_t[:]),
                        _mm_cast(g3sb[:]),
                        start=False,
                        stop=True,
                    )
                    pre_t = ew.tile([128, B], F32, tag="pre")
                    nc.vector.tensor_scalar_add(pre_t[:], pt[:], b1sb[:, m:m + 1])
                    update(col(m), pre_t, s2sb[:, col(m)], g2sb[:, col(m)])

                # phase C: layer-3 (output) update. pre3 = w2-matmul(g2) + b2
                pt3 = psum3.tile([D3, B], F32, tag="pt3")
                for k in range(NC1):
                    nc.tensor.matmul(
                        pt3[:],
                        _mm_cast(w2sb[:, k * D3:(k + 1) * D3]),
                        _mm_cast(g2sb[:, col(k)]),
                        start=(k == 0),
                        stop=(k == NC1 - 1),
                    )
                pre3 = ew.tile([D3, B], F32, tag="pre")
                nc.vector.tensor_scalar_add(pre3[:], pt3[:], b2sb[:])
                update(None, pre3, s3sb[:], g3sb[:])

            nc.sync.dma_start(out_d[:], s3sb[:])

    nc.compile()
    return nc


_NC_CACHE = {}


def _get_nc():
    if MM_MODE not in _NC_CACHE:
        _NC_CACHE[MM_MODE] = _build()
    return _NC_CACHE[MM_MODE]


def _prep_weights(w0, w1, w2, b0, b1, b2):
    np_mm = _np_mm_dt()
    w0p = np.ascontiguousarray(
        w0.reshape(NC0, 128, NC1, 128).transpose(2, 1, 0, 3).reshape(NC1, 128, D0)
    ).astype(np_mm)
    w1p = np.ascontiguousarray(
        w1.reshape(NC1, 128, NC1, 128).transpose(2, 1, 0, 3).reshape(NC1, 128, D1)
    ).astype(np_mm)
    w1tp = np.ascontiguousarray(
        w1.reshape(NC1, 128, NC1, 128).transpose(0, 3, 2, 1).reshape(NC1, 128, D1)
    ).astype(np_mm)
    w2p = np.ascontiguousarray(
        w2.reshape(NC1, 128, D3).transpose(1, 0, 2).reshape(128, NC1 * D3)
    ).astype(np_mm)
    w2tp = np.ascontiguousarray(w2.T).astype(np_mm)
    b0p = np.ascontiguousarray(b0.reshape(NC1, 128).T).astype(np.float32)
    b1p = np.ascontiguousarray(b1.reshape(NC1, 128).T).astype(np.float32)
    b2p = b2.reshape(D3, 1).astype(np.float32)
    return dict(w0p=w0p, w1p=w1p, w1tp=w1tp, w2p=w2p, w2tp=w2tp,
                b0p=b0p, b1p=b1p, b2p=b2p)


def _run(inputs, trace=False, trace_kwargs=None):
    x = np.asarray(inputs["x"], np.float32)
    s1 = np.asarray(inputs["s1"], np.float32)
    s2 = np.asarray(inputs["s2"], np.float32)
    s3 = np.asarray(inputs["s3"], np.float32)
    shared = _prep_weights(
        np.asarray(inputs["w0"], np.float32), np.asarray(inputs["w1"], np.float32),
        np.asarray(inputs["w2"], np.float32), np.asarray(inputs["b0"], np.float32),
        np.asarray(inputs["b1"], np.float32), np.asarray(inputs["b2"], np.float32))

    in_maps = []
    for c in range(N_CORES):
        rows = slice(c * B, (c + 1) * B)
        m = dict(shared)
        m["xT"] = np.ascontiguousarray(x[rows].T)
        m["s1t"] = np.ascontiguousarray(s1[rows].T)
        m["s2t"] = np.ascontiguousarray(s2[rows].T)
        m["s3t"] = np.ascontiguousarray(s3[rows].T)
        in_maps.append(m)

    nc = _get_nc()
    kw = {}
    if trace:
        kw = dict(trace=True, trace_kwargs=trace_kwargs or {})
    res = run_bass_kernel_spmd(nc, in_maps, list(range(N_CORES)), **kw)
    out = np.empty((BATCH, D3), np.float32)
    for c in range(N_CORES):
        out[c * B:(c + 1) * B, :] = res.results[c]["out"].T
    return out, res


def kernel(**inputs) -> np.ndarray:
    out, _ = _run(inputs)
    return out


def _make_in_maps(inputs):
    x = np.asarray(inputs["x"], np.float32)
    s1 = np.asarray(inputs["s1"], np.float32)
    s2 = np.asarray(inputs["s2"], np.float32)
    s3 = np.asarray(inputs["s3"], np.float32)
    shared = _prep_weights(
        np.asarray(inputs["w0"], np.float32), np.asarray(inputs["w1"], np.float32),
        np.asarray(inputs["w2"], np.float32), np.asarray(inputs["b0"], np.float32),
        np.asarray(inputs["b1"], np.float32), np.asarray(inputs["b2"], np.float32))
    in_maps = []
    for c in range(N_CORES):
        rows = slice(c * B, (c + 1) * B)
        m = dict(shared)
        m["xT"] = np.ascontiguousarray(x[rows].T)
        m["s1t"] = np.ascontiguousarray(s1[rows].T)
        m["s2t"] = np.ascontiguousarray(s2[rows].T)
        m["s3t"] = np.ascontiguousarray(s3[rows].T)
        in_maps.append(m)
    return in_maps


def timed_run(inputs, iters=5):
    """Run the kernel with device-resident inputs, timing each execution.

    Returns (output [4096,10], list of per-iteration wall seconds).
    """
    import time
    import jax
    from jax.sharding import Mesh, PartitionSpec, NamedSharding
    from jax.experimental.shard_map import shard_map
    from concourse import mybir as _mybir
    from concourse.bass2jax import _bass_exec_p, install_neuronx_cc_hook, partition_id_tensor

    install_neuronx_cc_hook()
    nc = _get_nc()
    in_maps = _make_in_maps(inputs)

    partition_name = nc.partition_id_tensor.name if nc.partition_id_tensor else None
    in_names, out_names, out_avals, zero_outs = [], [], [], []
    for alloc in nc.m.functions[0].allocations:
        if not isinstance(alloc, _mybir.MemoryLocationSet):
            continue
        name = alloc.memorylocations[0].name
        if alloc.kind == "ExternalInput":
            if name != partition_name:
                in_names.append(name)
        elif alloc.kind == "ExternalOutput":
            shape = tuple(alloc.tensor_shape)
            dtype = _mybir.dt.np(alloc.dtype)
            out_names.append(name)
            out_avals.append(jax.core.ShapedArray(shape, dtype))
            zero_outs.append(np.zeros(shape, dtype))
    n_params = len(in_names)
    all_in = list(in_names) + list(out_names)
    if partition_name is not None:
        all_in.append(partition_name)
    donate = tuple(range(n_params, n_params + len(out_names)))

    def _body(*args):
        operands = list(args)
        if partition_name is not None:
            operands.append(partition_id_tensor())
        outs = _bass_exec_p.bind(
            *operands,
            out_avals=tuple(out_avals),
            in_names=tuple(all_in),
            out_names=tuple(out_names),
            lowering_input_output_aliases=(),
            sim_require_finite=True,
            sim_require_nnan=True,
            nc=nc,
        )
        return tuple(outs)

    devices = jax.devices()[:N_CORES]
    mesh = Mesh(np.asarray(devices), ("core",))
    spec = PartitionSpec("core")
    sharded = jax.jit(
        shard_map(_body, mesh=mesh, in_specs=(spec,) * (n_params + len(out_names)),
                  out_specs=(spec,) * len(out_names), check_rep=False),
        donate_argnums=donate, keep_unused=True)

    concat_in = [
        np.concatenate([np.asarray(in_maps[c][nm]) for c in range(N_CORES)], axis=0)
        for nm in in_names
    ]
    sh = NamedSharding(mesh, spec)
    dev_in = [jax.device_put(a, sh) for a in concat_in]
    concat_zeros = [np.zeros((N_CORES * z.shape[0], *z.shape[1:]), z.dtype) for z in zero_outs]

    times = []
    out_arrs = None
    for it in range(iters + 1):
        zs = [jax.device_put(z, sh) for z in concat_zeros]
        jax.block_until_ready(zs)
        t0 = time.perf_counter()
        out_arrs = sharded(*dev_in, *zs)
        jax.block_until_ready(out_arrs)
        dt = time.perf_counter() - t0
        if it > 0:
            times.append(dt)

    res0 = np.asarray(out_arrs[0]).reshape(N_CORES, *out_avals[0].shape)
    out = np.empty((BATCH, D3), np.float32)
    for c in range(N_CORES):
        out[c * B:(c + 1) * B, :] = res0[c].T
    return out, times
